# revision 1
# baseline (speedup 1.0000x reference)
"""Trainium2 Bass kernel for nn_AdjointCircuitModule (13-qubit HEA circuit +
dense observable expectation), SPMD across 8 NeuronCores.

Strategy
--------
loss = <psi|O|psi> = a^T O a + b^T O b   (psi = a + i b, O real).

* The 2^13 = 8192 state is tiny, so every core simulates the FULL circuit
  redundantly (zero communication). State is held as an L0 matrix S[p, f]
  (p = qubits 0-6 on 128 partitions, f = qubits 7-12 on 64 free columns).
* Per hardware-efficient-ansatz layer (RX-all, RZ-all, CNOT-chain):
      U_layer = Pf . CX67 . (U_P (x) U_F)
      U_P = P7 . Dzp . H7 . Dxp . H7      (128x128 complex, runtime-built)
      U_F = Dzf . H6 . Dxf . (H6 Pf_prev) (64x64  complex)
  applied with transpose-fused TensorE matmuls:
      step A: T  = S^T @ U_P^T   (layout toggles L0 -> L1)
      step B: S' = T^T @ U_F^T   (L1 -> L0)
  CX67 (CNOT(6,7) spans the partition/free split) is done with per-partition
  masks + free-half swap on DVE.  Layer 8 folds its trailing free-CNOT-chain
  into step B via  Pf.CX67 = CX67'.Pf  with CX67' = E6 (x) I + O6 (x) FLIP6.
* The observable is row-sharded: core c streams O[1024c:1024c+1024, :]
  (32 MiB, 2 MiB tiles on a 10-deep ring so prefetch overlaps the circuit)
  through TensorE as the fp32r moving operand (full rate, M=2 psi-derived
  weights), accumulating v = O_rows^T [a b] in 8 PSUM banks per column
  phase.  Per-phase scalar_tensor_tensor ops with accum_out form the
  partial dot; the host sums the 8x128x2 partials.  The whole circuit runs
  in fp32r as well (2-4x fewer PE cycles than fp32).
"""

import math

import numpy as np

import concourse.bacc as bacc
import concourse.bass as bass
import concourse.mybir as mybir
import concourse.tile as tile
from concourse.bass_utils import run_bass_kernel_spmd

F32 = mybir.dt.float32
F32R = mybir.dt.float32r
AL = mybir.AluOpType
MV_DT = mybir.dt.float32r

N_CORES = 8
N_QUBITS = 13
N_LAYERS = 8
DIM = 2 ** N_QUBITS          # 8192
ROWS_PER_CORE = DIM // N_CORES   # 1024
N_PARAMS = 208

_CACHE = {}


# ----------------------------------------------------------------- host consts

def _cx_chain_perm(nq, pairs):
    dim = 2 ** nq
    P = np.eye(dim)
    for (c, t) in pairs:
        M = np.zeros((dim, dim))
        for i in range(dim):
            bc = (i >> (nq - 1 - c)) & 1
            j = i ^ ((1 << (nq - 1 - t)) if bc else 0)
            M[j, i] = 1.0
        P = M @ P
    return P


def _host_consts():
    H1 = np.array([[1, 1], [1, -1]], dtype=np.float64) / np.sqrt(2.0)

    def kron_n(n):
        out = np.array([[1.0]])
        for _ in range(n):
            out = np.kron(out, H1)
        return out

    H7 = kron_n(7)
    H6 = kron_n(6)
    P7 = _cx_chain_perm(7, [(q, q + 1) for q in range(6)])
    Pf = _cx_chain_perm(6, [(q, q + 1) for q in range(5)])

    # phase-accumulation sign matrices, with the /2 (RZ half-angle) and the
    # /(2 pi) (turns-based range reduction) folded in
    SGN7 = np.zeros((7, 128))
    for q in range(7):
        for p in range(128):
            b = (p >> (6 - q)) & 1
            SGN7[q, p] = (1.0 if b else -1.0) * 0.5 / (2.0 * math.pi)
    SGN6 = np.zeros((6, 64))
    for q in range(6):
        for f in range(64):
            b = (f >> (5 - q)) & 1
            SGN6[q, f] = (1.0 if b else -1.0) * 0.5 / (2.0 * math.pi)

    me = np.array([1.0 if (p % 2 == 0) else 0.0 for p in range(128)],
                  dtype=np.float64)[:, None]

    # pack every constant into one [128, 706] block (single DMA):
    # cols 0:128 h7 | 128:256 p7t | 256:320 h6 | 320:384 h6pf | 384:448 pft
    # | 448:512 i64 | 512:640 sgn7 (rows 0:7) | 640:704 sgn6 (rows 0:6)
    # | 704 me | 705 mo
    cons = np.zeros((128, 962), dtype=np.float64)
    cons[0, 706] = 1.0          # cols 706:770 = initial state e0; 770:834 = 0
    cons[:, 834:962] = np.eye(128)   # I128 (transpose identity)
    cons[:, 0:128] = H7
    cons[:, 128:256] = P7.T
    cons[0:64, 256:320] = H6
    cons[0:64, 320:384] = H6 @ Pf
    cons[0:64, 384:448] = Pf.T
    cons[0:64, 448:512] = np.eye(64)
    cons[0:7, 512:640] = SGN7
    cons[0:6, 640:704] = SGN6
    cons[:, 704:705] = me
    cons[:, 705:706] = 1.0 - me
    return {"cons": np.ascontiguousarray(cons, dtype=np.float32)}


# ------------------------------------------------------------------ the kernel

def _trig_tables(nc, tc, pool, phis_psum, npart, tag, width=32):
    """From r = phi/(2 pi) in PSUM [npart, 16] build SIN, COS, NSIN tables
    (sbuf [npart, 16]) of phi, robust to either cast-rounding semantics."""
    k_i = pool.tile([npart, width], mybir.dt.int32, tag=f"{tag}ki")
    nc.vector.tensor_copy(k_i[:, :], phis_psum)                  # cast f32->i32
    k_f = pool.tile([npart, width], F32, tag=f"{tag}kf")
    nc.vector.tensor_copy(k_f[:, :], k_i[:, :])                  # cast back
    frac = pool.tile([npart, width], F32, tag=f"{tag}fr")
    nc.vector.tensor_tensor(frac[:, :], phis_psum, k_f[:, :], AL.subtract)
    # frac in (-1, 1);  phi == 2*pi*frac (mod 2*pi)
    sh = pool.tile([npart, width], F32, tag=f"{tag}sh")             # sin(pi f)
    nc.scalar.activation(sh[:, :], frac[:, :],
                         mybir.ActivationFunctionType.Sin, scale=math.pi)
    u2 = pool.tile([npart, width], F32, tag=f"{tag}u2")             # sin(pi f/2)
    nc.scalar.activation(u2[:, :], frac[:, :],
                         mybir.ActivationFunctionType.Sin, scale=math.pi / 2)
    ch = pool.tile([npart, width], F32, tag=f"{tag}ch")             # cos(pi f)
    nc.vector.scalar_tensor_tensor(ch[:, :], u2[:, :], -2.0, u2[:, :],
                                   AL.mult, AL.mult)
    nc.vector.tensor_scalar_add(ch[:, :], ch[:, :], 1.0)
    sin = pool.tile([npart, width], F32, tag=f"{tag}sin")           # sin(2 pi f)
    nc.vector.scalar_tensor_tensor(sin[:, :], sh[:, :], 2.0, ch[:, :],
                                   AL.mult, AL.mult)
    cos = pool.tile([npart, width], F32, tag=f"{tag}cos")           # cos(2 pi f)
    nc.vector.scalar_tensor_tensor(cos[:, :], sh[:, :], -2.0, sh[:, :],
                                   AL.mult, AL.mult)
    nc.vector.tensor_scalar_add(cos[:, :], cos[:, :], 1.0)
    nsin = pool.tile([npart, width], F32, tag=f"{tag}ns")
    nc.vector.tensor_scalar_mul(nsin[:, :], sin[:, :], -1.0)
    return sin, cos, nsin


def _ap(x):
    return x if isinstance(x, bass.AP) else x[:, :]


def _build_program():
    nc = bacc.Bacc("TRN2", target_bir_lowering=False, debug=False,
                   num_devices=N_CORES)

    params = nc.dram_tensor("params", [N_PARAMS], F32, kind="ExternalInput")
    obs = nc.dram_tensor("obs", [ROWS_PER_CORE, DIM], F32, kind="ExternalInput")
    sel = nc.dram_tensor("sel", [64, 8], F32, kind="ExternalInput")
    cons = nc.dram_tensor("cons", [128, 962], F32, kind="ExternalInput")

    acc_out = nc.dram_tensor("acc", [128, 2], F32, kind="ExternalOutput")
    psi_re = nc.dram_tensor("psi_re", [DIM], F32, kind="ExternalOutput")
    psi_im = nc.dram_tensor("psi_im", [DIM], F32, kind="ExternalOutput")

    from contextlib import ExitStack
    with tile.TileContext(nc) as tc, ExitStack() as es:
        cpool = es.enter_context(tc.tile_pool(name="consts", bufs=1))
        wpool = es.enter_context(tc.tile_pool(name="wts", bufs=3))
        spool = es.enter_context(tc.tile_pool(name="state", bufs=2))
        opool = es.enter_context(tc.tile_pool(name="otiles", bufs=10))
        es_ps = ExitStack()   # circuit PSUM pools; closed before the matvec pool
        ppool_bld = es_ps.enter_context(tc.tile_pool(name="psbld", bufs=3, space="PSUM"))
        ppool_st = es_ps.enter_context(tc.tile_pool(name="psst", bufs=3, space="PSUM"))

        # ---- all constants arrive in one DMA; named views are AP slices
        cs = cpool.tile([128, 962], F32R, tag="cons")
        nc.sync.dma_start(cs[:, :], cons.ap().bitcast(F32R))
        h7 = cs[:, 0:128]
        p7t = cs[:, 128:256]
        h6 = cs[0:64, 256:320]
        h6pf = cs[0:64, 320:384]
        pft = cs[0:64, 384:448]
        i64 = cs[0:64, 448:512]
        sgn7 = cs[0:7, 512:640]
        sgn6 = cs[0:6, 640:704]
        me_ap = cs[:, 704:705].bitcast(F32)
        mo_ap = cs[:, 705:706].bitcast(F32)
        sel_sb = cpool.tile([64, 8], F32R, tag="sel")
        nc.sync.dma_start(sel_sb[:, :], sel.ap().bitcast(F32R))
        E_sb = cpool.tile([128, 2048], F32, tag="E")
        nc.vector.memset(E_sb[:, :], 0.0)
        v_all = cpool.tile([128, 2048], F32, tag="v_all")
        nc.vector.memset(v_all[:, :], 0.0)

        # ---- theta -> phase tables
        # params flat layout: k*26 + h*13 + q  (k layer, h 0=RX 1=RZ, q qubit)
        th_view = params.ap().rearrange("(k h q) -> q (k h)", k=8, h=2, q=13)
        thp = cpool.tile([7, 16], F32R, tag="thp")
        nc.sync.dma_start(thp[:, :], th_view[0:7].bitcast(F32R))
        thf = cpool.tile([6, 16], F32R, tag="thf")
        nc.sync.dma_start(thf[:, :], th_view[7:13].bitcast(F32R))

        phi_ps = ppool_bld.tile([128, 32], F32, tag="bld")
        nc.tensor.matmul(phi_ps[:, 0:16], sgn7, thp[:, :], start=True, stop=True)
        nc.tensor.matmul(phi_ps[0:64, 16:32], sgn6, thf[:, :], start=False,
                         stop=False, skip_group_check=True)
        nc.vector.memset(phi_ps[64:128, 16:32], 0.0)
        SIN_T, COS_T, NSIN_T = _trig_tables(nc, tc, cpool, phi_ps[:, :], 128, "t")
        SIN_P, COS_P, NSIN_P = SIN_T, COS_T, NSIN_T
        SIN_F = SIN_T[0:64, 16:32]
        COS_F = COS_T[0:64, 16:32]
        NSIN_F = NSIN_T[0:64, 16:32]

        # ---- initial state |0...0> straight from the const block
        i128 = cs[:, 834:962]
        a_cur = cs[:, 706:770]
        b_cur = cs[:, 770:834]
        bn_cur = cs[:, 770:834]

        for k in range(N_LAYERS):
            cxp = COS_P[:, 2 * k:2 * k + 1]
            sxp = SIN_P[:, 2 * k:2 * k + 1]
            czp = COS_P[:, 2 * k + 1:2 * k + 2]
            szp = SIN_P[:, 2 * k + 1:2 * k + 2]
            nszp = NSIN_P[:, 2 * k + 1:2 * k + 2]
            cxf = COS_F[:, 2 * k:2 * k + 1]
            sxf = SIN_F[:, 2 * k:2 * k + 1]
            czf = COS_F[:, 2 * k + 1:2 * k + 2]
            szf = SIN_F[:, 2 * k + 1:2 * k + 2]
            nszf = NSIN_F[:, 2 * k + 1:2 * k + 2]

            # ---- R_A = U_P^T as one wide [Re|Im] tile [128, 256]
            # M1 = H7 diag(cxp) H7, M2 = H7 diag(sxp) H7 (psum)
            # YRe = czp*M1 - szp*M2, YIm = szp*M1 + czp*M2 (z-rowscale)
            LC = wpool.tile([128, 128], F32R, tag="LC")
            nc.vector.tensor_scalar_mul(LC[:, :], h7, cxp)
            LS = wpool.tile([128, 128], F32R, tag="LS")
            nc.vector.tensor_scalar_mul(LS[:, :], h7, sxp)
            M1ps = ppool_bld.tile([128, 128], F32, tag="bld")
            nc.tensor.matmul(M1ps[:, :], LC[:, :], h7, start=True, stop=True)
            M2ps = ppool_bld.tile([128, 128], F32, tag="bld")
            nc.tensor.matmul(M2ps[:, :], LS[:, :], h7, start=True, stop=True)
            t1 = wpool.tile([128, 128], F32, tag="t1")
            nc.scalar.mul(t1[:, :], M1ps[:, :], czp)
            t2 = wpool.tile([128, 128], F32, tag="t2")
            nc.scalar.mul(t2[:, :], M1ps[:, :], szp)
            YRe = wpool.tile([128, 128], F32R, tag="YRe")
            nc.vector.scalar_tensor_tensor(YRe[:, :], M2ps[:, :], nszp, t1[:, :],
                                           AL.mult, AL.add)
            YIm = wpool.tile([128, 128], F32R, tag="YIm")
            nc.vector.scalar_tensor_tensor(YIm[:, :], M2ps[:, :], czp, t2[:, :],
                                           AL.mult, AL.add)
            RA = wpool.tile([128, 256], F32R, tag="RA")
            trRe = ppool_bld.tile([128, 128], F32R, tag="bld")
            nc.tensor.transpose(trRe[:, :], YRe[:, :], p7t)
            nc.scalar.copy(RA[:, 0:128], trRe[:, :])
            trIm = ppool_bld.tile([128, 128], F32R, tag="bld")
            nc.tensor.transpose(trIm[:, :], YIm[:, :], p7t)
            nc.vector.tensor_copy(RA[:, 128:256], trIm[:, :])

            # ---- R_B = U_F^T as one wide [Re|Im] tile [64, 128]
            rhsF = h6 if k == 0 else h6pf
            idF = pft if k == N_LAYERS - 1 else i64
            LCf = wpool.tile([64, 64], F32R, tag="LCf")
            nc.vector.tensor_scalar_mul(LCf[:, :], h6, cxf)
            LSf = wpool.tile([64, 64], F32R, tag="LSf")
            nc.vector.tensor_scalar_mul(LSf[:, :], h6, sxf)
            M1fps = ppool_bld.tile([64, 64], F32, tag="bld")
            nc.tensor.matmul(M1fps[:, :], LCf[:, :], rhsF, start=True, stop=True)
            M2fps = ppool_bld.tile([64, 64], F32, tag="bld")
            nc.tensor.matmul(M2fps[:, :], LSf[:, :], rhsF, start=True, stop=True)
            t1f = wpool.tile([64, 64], F32, tag="t1f")
            nc.scalar.mul(t1f[:, :], M1fps[:, :], czf)
            t2f = wpool.tile([64, 64], F32, tag="t2f")
            nc.scalar.mul(t2f[:, :], M1fps[:, :], szf)
            YFRe = wpool.tile([64, 64], F32R, tag="YFRe")
            nc.vector.scalar_tensor_tensor(YFRe[:, :], M2fps[:, :], nszf,
                                           t1f[:, :], AL.mult, AL.add)
            YFIm = wpool.tile([64, 64], F32R, tag="YFIm")
            nc.vector.scalar_tensor_tensor(YFIm[:, :], M2fps[:, :], czf,
                                           t2f[:, :], AL.mult, AL.add)
            RB = wpool.tile([64, 128], F32R, tag="RB")
            trFRe = ppool_bld.tile([64, 64], F32R, tag="bld")
            nc.tensor.transpose(trFRe[:, :], YFRe[:, :], idF)
            nc.scalar.copy(RB[:, 0:64], trFRe[:, :])
            trFIm = ppool_bld.tile([64, 64], F32R, tag="bld")
            nc.tensor.transpose(trFIm[:, :], YFIm[:, :], idF)
            nc.vector.tensor_copy(RB[:, 64:128], trFIm[:, :])

            # ---- step A: psA = [re|im] = [a^T Re - b^T Im | a^T Im + b^T Re]
            psA = ppool_st.tile([64, 256], F32, tag="st")
            nc.tensor.matmul(psA[:, :], _ap(a_cur), RA[:, :],
                             start=True, stop=True)
            nc.tensor.matmul(psA[:, 0:128], _ap(bn_cur), RA[:, 128:256],
                             start=False, stop=False, skip_group_check=True)
            nc.tensor.matmul(psA[:, 128:256], _ap(b_cur), RA[:, 0:128],
                             start=False, stop=False, skip_group_check=True)
            aL1 = spool.tile([64, 128], F32R, tag="aL1")
            nc.scalar.copy(aL1[:, :], psA[:, 0:128])
            bL1 = spool.tile([64, 128], F32R, tag="bL1")
            nc.vector.tensor_copy(bL1[:, :], psA[:, 128:256])
            bL1n = spool.tile([64, 128], F32R, tag="bL1n")
            nc.scalar.mul(bL1n[:, :], psA[:, 128:256], -1.0)

            # ---- step B
            psB = ppool_st.tile([128, 128], F32, tag="st")
            nc.tensor.matmul(psB[:, :], aL1[:, :], RB[:, :],
                             start=True, stop=True)
            nc.tensor.matmul(psB[:, 0:64], bL1n[:, :], RB[:, 64:128],
                             start=False, stop=False, skip_group_check=True)
            nc.tensor.matmul(psB[:, 64:128], bL1[:, :], RB[:, 0:64],
                             start=False, stop=False, skip_group_check=True)
            a2 = spool.tile([128, 64], F32, tag="a2")
            nc.scalar.copy(a2[:, :], psB[:, 0:64])
            b2 = spool.tile([128, 64], F32, tag="b2")
            nc.vector.tensor_copy(b2[:, :], psB[:, 64:128])

            # ---- CX67 (CNOT(6,7)); layer 8 uses the folded CX67' (FLIP6)
            ta = spool.tile([128, 64], F32, tag="ta")
            nc.scalar.mul(ta[:, :], a2[:, :], mo_ap)
            tb = spool.tile([128, 64], F32, tag="tb")
            nc.scalar.mul(tb[:, :], b2[:, :], mo_ap)
            a3 = spool.tile([128, 64], F32R, tag="sa")
            b3 = spool.tile([128, 64], F32R, tag="sb")
            if k < N_LAYERS - 1:
                for src, t, dst in ((a2, ta, a3), (b2, tb, b3)):
                    nc.vector.scalar_tensor_tensor(
                        dst[:, 0:32], src[:, 0:32], me_ap, t[:, 32:64],
                        AL.mult, AL.add)
                    nc.vector.scalar_tensor_tensor(
                        dst[:, 32:64], src[:, 32:64], me_ap, t[:, 0:32],
                        AL.mult, AL.add)
            else:
                nc.vector.scalar_tensor_tensor(
                    a3[:, :], a2[:, :], me_ap, ta[:, ::-1], AL.mult, AL.add)
                nc.vector.scalar_tensor_tensor(
                    b3[:, :], b2[:, :], me_ap, tb[:, ::-1], AL.mult, AL.add)
            bn3 = spool.tile([128, 64], F32R, tag="sbn")
            nc.vector.tensor_scalar_mul(bn3[:, :], b3[:, :], -1.0)
            a_cur, b_cur, bn_cur = a3, b3, bn3

        # ---- psi -> DRAM (outputs double as rearrangement scratch)
        nc.sync.dma_start(psi_re.ap().rearrange("(p f) -> p f", p=128)
                           .bitcast(F32R), _ap(a_cur))
        nc.sync.dma_start(psi_im.ap().rearrange("(p f) -> p f", p=128)
                           .bitcast(F32R), _ap(b_cur))
        # E_sb[32g+comp, 512t+u] = psi_comp[512(4t+g)+u]; matches the matvec
        # drain layout (chunk n=4t+g at partition base 32g, col chunk t).
        # Engine APs may only start at partition 0/32/64/96.
        for g in range(4):
            for comp, src in ((0, psi_re), (1, psi_im)):
                src_ap = bass.AP(src.ap().tensor, 512 * g, [[2048, 4], [1, 512]])
                nc.sync.dma_start(E_sb[32 * g + comp:32 * g + comp + 1, :]
                                    .rearrange("p (t u) -> p t u", t=4), src_ap)
        # A64[r, 64*chi + clo] = psi[128 r + 64 chi + clo], built on-chip:
        # aT = transpose(state); A64 block chi = transpose(aT[:, chi::2])
        A64a = cpool.tile([64, 128], F32R, tag="A64a")
        A64b = cpool.tile([64, 128], F32R, tag="A64b")
        for comp, (st, A64) in enumerate(((a_cur, A64a), (b_cur, A64b))):
            tps = ppool_bld.tile([64, 128], F32R, tag="bld", name=f"tps{comp}")
            nc.tensor.transpose(tps[:, :], _ap(st), i128)
            aTs = cpool.tile([64, 128], F32R, tag=f"aTs{comp}")
            if comp == 0:
                nc.scalar.copy(aTs[:, :], tps[:, :])
            else:
                nc.vector.tensor_copy(aTs[:, :], tps[:, :])
            for chi in range(2):
                tr2 = ppool_bld.tile([64, 64], F32R, tag="bld",
                                     name=f"tr2{comp}{chi}")
                src = aTs[:, :].rearrange("p (c two) -> two p c", two=2)[chi]
                nc.tensor.transpose(tr2[:, :], src, i64)
                if chi == 0:
                    nc.scalar.copy(A64[:, 0:64], tr2[:, :])
                else:
                    nc.vector.tensor_copy(A64[:, 64:128], tr2[:, :])

        # ---- W[k, 2t]=a[1024c+128t+k], W[k, 2t+1]=b[...]: A64^T @ Sel
        psWa = ppool_st.tile([128, 8], F32, tag="st")
        nc.tensor.matmul(psWa[:, :], A64a[:, :], sel_sb[:, :], start=True, stop=True)
        psWb = ppool_st.tile([128, 8], F32, tag="st")
        nc.tensor.matmul(psWb[:, :], A64b[:, :], sel_sb[:, :], start=True, stop=True)
        W = cpool.tile([128, 16], MV_DT, tag="W")
        wv = W[:, :].rearrange("p (t two) -> two p t", two=2)
        nc.vector.tensor_copy(wv[0], psWa[:, :])
        nc.vector.tensor_copy(wv[1], psWb[:, :])

        # ---- matvec: stream O shard in 2 column-phases of 4096; within a
        #      phase, 8 n-chunks accumulate in 8 PSUM banks (fp32r needs
        #      psum partition base 0), K-accumulated over the 8 row-chunks.
        es_ps.close()   # release circuit PSUM banks
        ppool_mv = es.enter_context(tc.tile_pool(name="psmv", bufs=1,
                                                 space="PSUM"))
        acc_sb = cpool.tile([128, 2], F32, tag="acc")
        for phase in range(2):
            PS = [ppool_mv.tile([2, 512], F32, tag=f"P{j}", name=f"P{j}")
                  for j in range(8)]
            for rc in range(8):
                ot = opool.tile([128, DIM // 2], MV_DT, tag="ot")
                nc.sync.dma_start(
                    ot[:, :],
                    obs.ap()[128 * rc:128 * rc + 128,
                             4096 * phase:4096 * (phase + 1)].bitcast(MV_DT))
                wsl = W[:, 2 * rc:2 * rc + 2]
                for j in range(8):
                    nc.tensor.matmul(PS[j][:, :], wsl,
                                     ot[:, 512 * j:512 * (j + 1)],
                                     start=(rc == 0), stop=(rc == 7))
            for j in range(8):
                n = 8 * phase + j
                g, t = n % 4, n // 4
                dst = v_all[32 * g:32 * g + 2, 512 * t:512 * (t + 1)]
                if j % 2 == 0:
                    nc.scalar.copy(dst, PS[j][:, :])
                else:
                    nc.vector.tensor_copy(dst, PS[j][:, :])
            # partial epilogue for this phase's column range (overlaps phase 1)
            cl = slice(1024 * phase, 1024 * (phase + 1))
            nc.vector.scalar_tensor_tensor(v_all[:, cl], v_all[:, cl], 0.0,
                                           E_sb[:, cl], AL.bypass, AL.mult,
                                           accum_out=acc_sb[:, phase:phase + 1])
        nc.sync.dma_start(acc_out.ap(), acc_sb[:, :])

    nc.compile()
    return nc


def _get_program():
    if "nc" not in _CACHE:
        _CACHE["nc"] = _build_program()
        _CACHE["consts"] = _host_consts()
    return _CACHE["nc"], _CACHE["consts"]


def _make_in_maps(params, observable):
    nc, consts = _get_program()
    params = np.ascontiguousarray(params, dtype=np.float32)
    observable = np.asarray(observable, dtype=np.float32)
    eye64 = np.eye(64, dtype=np.float32)
    in_maps = []
    for c in range(N_CORES):
        m = dict(consts)
        m["params"] = params
        m["obs"] = np.ascontiguousarray(
            observable[c * ROWS_PER_CORE:(c + 1) * ROWS_PER_CORE, :])
        m["sel"] = np.ascontiguousarray(eye64[:, 8 * c:8 * c + 8])
        in_maps.append(m)
    return nc, in_maps


def run(params, observable, trace=False):
    nc, in_maps = _make_in_maps(params, observable)
    res = run_bass_kernel_spmd(nc, in_maps, core_ids=list(range(N_CORES)),
                               trace=trace)
    loss = np.float32(sum(float(r["acc"].sum()) for r in res.results))
    return loss, res


def kernel(params, observable):
    loss, _ = run(params, observable, trace=False)
    return np.float32(loss)



# revision 6
# speedup vs baseline: 1.7244x; 1.7244x over previous
"""Trainium2 Bass kernel for nn_AdjointCircuitModule (13-qubit HEA circuit +
dense observable expectation), SPMD across 8 NeuronCores.

Strategy (v2)
-------------
loss = <psi|O|psi> = psi^T Osym psi, Osym = (O + O^T)/2.  Only the symmetric
part matters, so the host streams the upper triangle of S = O + O^T in
512x512 blocks: 136 blocks, 17 per core (every block identical cost =>
perfectly uniform SPMD program).  Blocks are sent as bf16 (error ~2e-3 on
the scalar), halving bytes again: 8.9 MB/core vs 32 MB full-f32.

* Circuit: every core simulates the full 13-qubit circuit redundantly.
  State held as L0 matrix S[p, f] (qubits 0-6 on 128 partitions, 7-12 on 64
  free cols).  Per layer U = Pf . CX67 . (U_P (x) U_F):
    - all per-layer weight tiles (RAW = [RAre|RAim|-RAim] for step A,
      RBW = [RBre|RBdre|RBim|RBdim|-RBim|-RBdim] for step B) are built
      from trig tables up front / pipelined across layers,
    - state chain per layer: psA = 3 matmuls, 1 wide PSUM->SBUF copy
      (split on 2 engines), psB = 3 matmuls (extra _d columns compute the
      CX67 column-flip difference), then CX67 = 2 scalar_tensor_tensor ops
      (S'' = S' + mo * (S'[flip] - S')).
* Matvec: per block g (17 groups): PSUM [2,512] accumulates 4 matmuls
  (stationary = psi rows of the block as bf16 pairs from W, moving = the
  bf16 block tile straight from the SBUF-resident stream).  Groups drain
  to SBUF vout; one DMA returns [2, 8704] per core and the host does the
  final 17x512-dot against psi (psi_re/psi_im are outputs too).
"""

import math

import numpy as np
import ml_dtypes

import concourse.bacc as bacc
import concourse.bass as bass
import concourse.mybir as mybir
import concourse.tile as tile
from concourse.bass_utils import run_bass_kernel_spmd

F32 = mybir.dt.float32
F32R = mybir.dt.float32r
BF16 = mybir.dt.bfloat16
AL = mybir.AluOpType

N_CORES = 8
N_QUBITS = 13
N_LAYERS = 8
DIM = 2 ** N_QUBITS          # 8192
N_PARAMS = 208
NBLK = 17                    # 512x512 blocks per core
BLK = 512
STREAM_COLS = NBLK * 4 * BLK   # 34816

_CACHE = {}

# 17 blocks per core out of the 136 upper-triangle (R <= C) 512-blocks
BLOCKS = [(R, C) for R in range(16) for C in range(R, 16)]


# ----------------------------------------------------------------- host consts

def _cx_chain_perm(nq, pairs):
    dim = 2 ** nq
    P = np.eye(dim)
    for (c, t) in pairs:
        M = np.zeros((dim, dim))
        for i in range(dim):
            bc = (i >> (nq - 1 - c)) & 1
            j = i ^ ((1 << (nq - 1 - t)) if bc else 0)
            M[j, i] = 1.0
        P = M @ P
    return P


def _host_consts():
    H1 = np.array([[1, 1], [1, -1]], dtype=np.float64) / np.sqrt(2.0)

    def kron_n(n):
        out = np.array([[1.0]])
        for _ in range(n):
            out = np.kron(out, H1)
        return out

    H7 = kron_n(7)
    H6 = kron_n(6)
    P7 = _cx_chain_perm(7, [(q, q + 1) for q in range(6)])
    Pf = _cx_chain_perm(6, [(q, q + 1) for q in range(5)])

    # phase-accumulation sign matrices, with the /2 (RZ half-angle) and the
    # /(2 pi) (turns-based range reduction) folded in
    SGN7 = np.zeros((7, 128))
    for q in range(7):
        for p in range(128):
            b = (p >> (6 - q)) & 1
            SGN7[q, p] = (1.0 if b else -1.0) * 0.5 / (2.0 * math.pi)
    SGN6 = np.zeros((6, 64))
    for q in range(6):
        for f in range(64):
            b = (f >> (5 - q)) & 1
            SGN6[q, f] = (1.0 if b else -1.0) * 0.5 / (2.0 * math.pi)

    mo = np.array([0.0 if (p % 2 == 0) else 1.0 for p in range(128)],
                  dtype=np.float64)[:, None]

    # pack every constant into one [128, 962] block (single DMA):
    # cols 0:128 h7 | 128:256 p7t | 256:320 h6 | 320:384 h6pf | 384:448 pft
    # | 448:512 i64 | 512:640 sgn7 (rows 0:7) | 640:704 sgn6 (rows 0:6)
    # | 704 me | 705 mo | 706:770 e0 | 770:834 zeros | 834:962 I128
    cons = np.zeros((128, 962), dtype=np.float64)
    cons[0, 706] = 1.0
    cons[:, 834:962] = np.eye(128)
    cons[:, 0:128] = H7
    cons[:, 128:256] = P7.T
    cons[0:64, 256:320] = H6
    cons[0:64, 320:384] = H6 @ Pf
    cons[0:64, 384:448] = Pf.T
    cons[0:64, 448:512] = np.eye(64)
    cons[0:7, 512:640] = SGN7
    cons[0:6, 640:704] = SGN6
    cons[:, 704:705] = 1.0 - mo
    cons[:, 705:706] = mo
    return {"cons": np.ascontiguousarray(cons, dtype=np.float32)}


# ------------------------------------------------------------------ the kernel

def _trig_tables(nc, tc, pool, phis_psum, npart, tag, width=32):
    """From r = phi/(2 pi) in PSUM [npart, 16] build SIN, COS, NSIN tables
    (sbuf [npart, 16]) of phi, robust to either cast-rounding semantics."""
    k_i = pool.tile([npart, width], mybir.dt.int32, tag=f"{tag}ki")
    nc.vector.tensor_copy(k_i[:, :], phis_psum)                  # cast f32->i32
    k_f = pool.tile([npart, width], F32, tag=f"{tag}kf")
    nc.vector.tensor_copy(k_f[:, :], k_i[:, :])                  # cast back
    frac = pool.tile([npart, width], F32, tag=f"{tag}fr")
    nc.vector.tensor_tensor(frac[:, :], phis_psum, k_f[:, :], AL.subtract)
    # frac in (-1, 1);  phi == 2*pi*frac (mod 2*pi)
    sh = pool.tile([npart, width], F32, tag=f"{tag}sh")             # sin(pi f)
    nc.scalar.activation(sh[:, :], frac[:, :],
                         mybir.ActivationFunctionType.Sin, scale=math.pi)
    u2 = pool.tile([npart, width], F32, tag=f"{tag}u2")             # sin(pi f/2)
    nc.scalar.activation(u2[:, :], frac[:, :],
                         mybir.ActivationFunctionType.Sin, scale=math.pi / 2)
    ch = pool.tile([npart, width], F32, tag=f"{tag}ch")             # cos(pi f)
    nc.vector.scalar_tensor_tensor(ch[:, :], u2[:, :], -2.0, u2[:, :],
                                   AL.mult, AL.mult)
    nc.vector.tensor_scalar_add(ch[:, :], ch[:, :], 1.0)
    sin = pool.tile([npart, width], F32, tag=f"{tag}sin")           # sin(2 pi f)
    nc.vector.scalar_tensor_tensor(sin[:, :], sh[:, :], 2.0, ch[:, :],
                                   AL.mult, AL.mult)
    cos = pool.tile([npart, width], F32, tag=f"{tag}cos")           # cos(2 pi f)
    nc.vector.scalar_tensor_tensor(cos[:, :], sh[:, :], -2.0, sh[:, :],
                                   AL.mult, AL.mult)
    nc.vector.tensor_scalar_add(cos[:, :], cos[:, :], 1.0)
    nsin = pool.tile([npart, width], F32, tag=f"{tag}ns")
    nc.vector.tensor_scalar_mul(nsin[:, :], sin[:, :], -1.0)
    return sin, cos, nsin


def _ap(x):
    return x if isinstance(x, bass.AP) else x[:, :]


def _build_program():
    nc = bacc.Bacc("TRN2", target_bir_lowering=False, debug=False,
                   num_devices=N_CORES)

    params = nc.dram_tensor("params", [N_PARAMS], F32, kind="ExternalInput")
    obs = nc.dram_tensor("obs", [128, STREAM_COLS], BF16, kind="ExternalInput")
    sel = nc.dram_tensor("sel", [64, 4 * NBLK], F32, kind="ExternalInput")
    cons = nc.dram_tensor("cons", [128, 962], F32, kind="ExternalInput")

    vout = nc.dram_tensor("vout", [2, NBLK * BLK], F32, kind="ExternalOutput")
    psi_re = nc.dram_tensor("psi_re", [DIM], F32, kind="ExternalOutput")
    psi_im = nc.dram_tensor("psi_im", [DIM], F32, kind="ExternalOutput")

    from contextlib import ExitStack
    with tile.TileContext(nc) as tc, ExitStack() as es:
        cpool = es.enter_context(tc.tile_pool(name="consts", bufs=1))
        wpool = es.enter_context(tc.tile_pool(name="wts", bufs=3))
        spool = es.enter_context(tc.tile_pool(name="state", bufs=3))
        opool = es.enter_context(tc.tile_pool(name="otiles", bufs=1))
        es_ps = ExitStack()   # circuit PSUM pools; closed before the matvec pool
        ppool_bld = es_ps.enter_context(tc.tile_pool(name="psbld", bufs=2, space="PSUM"))
        ppool_st = es_ps.enter_context(tc.tile_pool(name="psst", bufs=2, space="PSUM"))

        # ---- control-plane DMAs first, then the 17 obs block-group chunks
        cs = cpool.tile([128, 962], F32R, tag="cons")
        nc.sync.dma_start(cs[:, :], cons.ap().bitcast(F32R))
        sel_sb = cpool.tile([64, 4 * NBLK], F32R, tag="sel")
        nc.sync.dma_start(sel_sb[:, :], sel.ap().bitcast(F32R))
        # params flat layout: k*26 + h*13 + q  (k layer, h 0=RX 1=RZ, q qubit)
        th_view = params.ap().rearrange("(k h q) -> q (k h)", k=8, h=2, q=13)
        thp = cpool.tile([7, 16], F32R, tag="thp")
        nc.sync.dma_start(thp[:, :], th_view[0:7].bitcast(F32R))
        thf = cpool.tile([6, 16], F32R, tag="thf")
        nc.sync.dma_start(thf[:, :], th_view[7:13].bitcast(F32R))

        obs_t = []
        for g in range(NBLK):
            ot = opool.tile([128, 4 * BLK], BF16, tag=f"obs{g}")
            nc.sync.dma_start(ot[:, :],
                              obs.ap()[:, 4 * BLK * g:4 * BLK * (g + 1)])
            obs_t.append(ot)

        # ---- named views into the const block
        h7 = cs[:, 0:128]
        p7t = cs[:, 128:256]
        h6 = cs[0:64, 256:320]
        h6pf = cs[0:64, 320:384]
        pft = cs[0:64, 384:448]
        i64 = cs[0:64, 448:512]
        sgn7 = cs[0:7, 512:640]
        sgn6 = cs[0:6, 640:704]
        mo_ap = cs[:, 705:706].bitcast(F32)
        i128 = cs[:, 834:962]
        a0 = cs[:, 706:770]

        # ---- theta -> phase tables
        phi_ps = ppool_bld.tile([128, 32], F32, tag="bld")
        nc.tensor.matmul(phi_ps[:, 0:16], sgn7, thp[:, :], start=True, stop=True)
        nc.tensor.matmul(phi_ps[0:64, 16:32], sgn6, thf[:, :], start=False,
                         stop=False, skip_group_check=True)
        nc.vector.memset(phi_ps[64:128, 16:32], 0.0)
        SIN_T, COS_T, NSIN_T = _trig_tables(nc, tc, cpool, phi_ps[:, :], 128, "t")
        SIN_P, COS_P, NSIN_P = SIN_T, COS_T, NSIN_T
        SIN_F = SIN_T[0:64, 16:32]
        COS_F = COS_T[0:64, 16:32]
        NSIN_F = NSIN_T[0:64, 16:32]

        # persistent per-layer weight tiles
        RAW = [cpool.tile([128, 384], F32R, tag=f"RAW{k}", name=f"RAW{k}")
               for k in range(N_LAYERS)]
        RBW = [cpool.tile([64, 384], F32R, tag=f"RBW{k}", name=f"RBW{k}")
               for k in range(N_LAYERS)]

        def build_layer(k):
            cxp = COS_P[:, 2 * k:2 * k + 1]
            sxp = SIN_P[:, 2 * k:2 * k + 1]
            czp = COS_P[:, 2 * k + 1:2 * k + 2]
            szp = SIN_P[:, 2 * k + 1:2 * k + 2]
            nszp = NSIN_P[:, 2 * k + 1:2 * k + 2]
            cxf = COS_F[:, 2 * k:2 * k + 1]
            sxf = SIN_F[:, 2 * k:2 * k + 1]
            czf = COS_F[:, 2 * k + 1:2 * k + 2]
            szf = SIN_F[:, 2 * k + 1:2 * k + 2]
            nszf = NSIN_F[:, 2 * k + 1:2 * k + 2]

            # ---- RAW_k = [RAre | RAim | -RAim],  RA = U_P^T
            LC = wpool.tile([128, 128], F32R, tag="LC")
            nc.vector.tensor_scalar_mul(LC[:, :], h7, cxp)
            LS = wpool.tile([128, 128], F32R, tag="LS")
            nc.vector.tensor_scalar_mul(LS[:, :], h7, sxp)
            M1ps = ppool_bld.tile([128, 128], F32, tag="bld")
            nc.tensor.matmul(M1ps[:, :], LC[:, :], h7, start=True, stop=True)
            M2ps = ppool_bld.tile([128, 128], F32, tag="bld")
            nc.tensor.matmul(M2ps[:, :], LS[:, :], h7, start=True, stop=True)
            t1 = wpool.tile([128, 128], F32, tag="t1")
            nc.scalar.mul(t1[:, :], M1ps[:, :], czp)
            t2 = wpool.tile([128, 128], F32, tag="t2")
            nc.scalar.mul(t2[:, :], M1ps[:, :], szp)
            YRe = wpool.tile([128, 128], F32R, tag="YRe")
            nc.vector.scalar_tensor_tensor(YRe[:, :], M2ps[:, :], nszp, t1[:, :],
                                           AL.mult, AL.add)
            YIm = wpool.tile([128, 128], F32R, tag="YIm")
            nc.vector.scalar_tensor_tensor(YIm[:, :], M2ps[:, :], czp, t2[:, :],
                                           AL.mult, AL.add)
            trRe = ppool_bld.tile([128, 128], F32R, tag="bld")
            nc.tensor.transpose(trRe[:, :], YRe[:, :], p7t)
            trIm = ppool_bld.tile([128, 128], F32R, tag="bld")
            nc.tensor.transpose(trIm[:, :], YIm[:, :], p7t)
            raw = RAW[k]
            nc.scalar.copy(raw[:, 0:128], trRe[:, :])
            nc.vector.tensor_copy(raw[:, 128:256], trIm[:, :])
            nc.scalar.mul(raw[:, 256:384], trIm[:, :], -1.0)

            # ---- RBW_k = [RBre | RBdre | RBim | RBdim | -RBim | -RBdim]
            rhsF = h6 if k == 0 else h6pf
            idF = pft if k == N_LAYERS - 1 else i64
            LCf = wpool.tile([64, 64], F32R, tag="LCf")
            nc.vector.tensor_scalar_mul(LCf[:, :], h6, cxf)
            LSf = wpool.tile([64, 64], F32R, tag="LSf")
            nc.vector.tensor_scalar_mul(LSf[:, :], h6, sxf)
            M1fps = ppool_bld.tile([64, 64], F32, tag="bldf")
            nc.tensor.matmul(M1fps[:, :], LCf[:, :], rhsF, start=True, stop=True)
            M2fps = ppool_bld.tile([64, 64], F32, tag="bldf")
            nc.tensor.matmul(M2fps[:, :], LSf[:, :], rhsF, start=True, stop=True)
            t1f = wpool.tile([64, 64], F32, tag="t1f")
            nc.scalar.mul(t1f[:, :], M1fps[:, :], czf)
            t2f = wpool.tile([64, 64], F32, tag="t2f")
            nc.scalar.mul(t2f[:, :], M1fps[:, :], szf)
            YFRe = wpool.tile([64, 64], F32R, tag="YFRe")
            nc.vector.scalar_tensor_tensor(YFRe[:, :], M2fps[:, :], nszf,
                                           t1f[:, :], AL.mult, AL.add)
            YFIm = wpool.tile([64, 64], F32R, tag="YFIm")
            nc.vector.scalar_tensor_tensor(YFIm[:, :], M2fps[:, :], czf,
                                           t2f[:, :], AL.mult, AL.add)
            trFRe = ppool_bld.tile([64, 64], F32R, tag="bldf")
            nc.tensor.transpose(trFRe[:, :], YFRe[:, :], idF)
            trFIm = ppool_bld.tile([64, 64], F32R, tag="bldf")
            nc.tensor.transpose(trFIm[:, :], YFIm[:, :], idF)
            rbw = RBW[k]
            # CX67 flip on the free axis: half-swap (f ^ 32) for k<7, the
            # folded layer-8 variant is a full reversal.
            if k < N_LAYERS - 1:
                def fv(t):   # [64, 2, 32] view with the 32-col halves swapped
                    v = t.rearrange("p (b x) -> p b x", b=2)
                    return v[:, ::-1, :]

                def dv(t):
                    return t.rearrange("p (b x) -> p b x", b=2)
            else:
                def fv(t):
                    return t[:, ::-1]

                def dv(t):
                    return t
            nc.scalar.copy(rbw[:, 0:64], trFRe[:, :])
            nc.vector.tensor_tensor(dv(rbw[:, 64:128]), fv(rbw[:, 0:64]),
                                    dv(rbw[:, 0:64]), AL.subtract)
            nc.scalar.copy(rbw[:, 128:192], trFIm[:, :])
            nc.vector.tensor_tensor(dv(rbw[:, 192:256]), fv(rbw[:, 128:192]),
                                    dv(rbw[:, 128:192]), AL.subtract)
            nc.scalar.mul(rbw[:, 256:320], trFIm[:, :], -1.0)
            nc.vector.tensor_scalar_mul(rbw[:, 320:384], rbw[:, 192:256], -1.0)

        def chain_layer(k, a_cur, b_cur):
            raw, rbw = RAW[k], RBW[k]
            psA = ppool_st.tile([64, 256], F32, tag="stA")
            nc.tensor.matmul(psA[:, :], _ap(a_cur), raw[:, 0:256],
                             start=True, stop=True)
            if k > 0:
                nc.tensor.matmul(psA[:, 0:128], _ap(b_cur), raw[:, 256:384],
                                 start=False, stop=False, skip_group_check=True)
                nc.tensor.matmul(psA[:, 128:256], _ap(b_cur), raw[:, 0:128],
                                 start=False, stop=False, skip_group_check=True)
            T1 = spool.tile([64, 256], F32R, tag="T1")
            nc.scalar.copy(T1[:, 0:128], psA[:, 0:128])
            nc.vector.tensor_copy(T1[:, 128:256], psA[:, 128:256])

            psB = ppool_st.tile([128, 256], F32, tag="stB")
            nc.tensor.matmul(psB[:, :], T1[:, 0:128], rbw[:, 0:256],
                             start=True, stop=True)
            nc.tensor.matmul(psB[:, 0:128], T1[:, 128:256], rbw[:, 256:384],
                             start=False, stop=False, skip_group_check=True)
            nc.tensor.matmul(psB[:, 128:256], T1[:, 128:256], rbw[:, 0:128],
                             start=False, stop=False, skip_group_check=True)
            keepT = spool.tile([128, 128], F32R, tag="keepT")
            nc.scalar.copy(
                keepT[:, :].rearrange("p (two x) -> p two x", two=2),
                psB[:, :].rearrange("p (four x) -> p four x", four=4)[:, 0::2, :])
            a3 = spool.tile([128, 64], F32R, tag="a3")
            nc.vector.scalar_tensor_tensor(a3[:, :], psB[:, 64:128], mo_ap,
                                           keepT[:, 0:64], AL.mult, AL.add)
            b3 = spool.tile([128, 64], F32R, tag="b3")
            nc.vector.scalar_tensor_tensor(b3[:, :], psB[:, 192:256], mo_ap,
                                           keepT[:, 64:128], AL.mult, AL.add)
            return a3, b3

        # interleave: builds run ahead so PE stays busy during chain stalls
        a_cur, b_cur = a0, None
        build_layer(0)
        build_layer(1)
        for k in range(N_LAYERS):
            if k + 2 < N_LAYERS:
                build_layer(k + 2)
            a_cur, b_cur = chain_layer(k, a_cur, b_cur)

        # ---- psi -> DRAM (host uses it for the final dot)
        nc.sync.dma_start(psi_re.ap().rearrange("(p f) -> p f", p=128)
                           .bitcast(F32R), _ap(a_cur))
        nc.sync.dma_start(psi_im.ap().rearrange("(p f) -> p f", p=128)
                           .bitcast(F32R), _ap(b_cur))

        # ---- A64[r, 64*chi + clo] = psi[128 r + 64 chi + clo]
        A64a = cpool.tile([64, 128], F32R, tag="A64a")
        A64b = cpool.tile([64, 128], F32R, tag="A64b")
        for comp, (st, A64) in enumerate(((a_cur, A64a), (b_cur, A64b))):
            tps = ppool_bld.tile([64, 128], F32R, tag="bld", name=f"tps{comp}")
            nc.tensor.transpose(tps[:, :], _ap(st), i128)
            aTs = cpool.tile([64, 128], F32R, tag=f"aTs{comp}")
            if comp == 0:
                nc.scalar.copy(aTs[:, :], tps[:, :])
            else:
                nc.vector.tensor_copy(aTs[:, :], tps[:, :])
            for chi in range(2):
                tr2 = ppool_bld.tile([64, 64], F32R, tag="bldf",
                                     name=f"tr2{comp}{chi}")
                src = aTs[:, :].rearrange("p (c two) -> two p c", two=2)[chi]
                nc.tensor.transpose(tr2[:, :], src, i64)
                if chi == 0:
                    nc.scalar.copy(A64[:, 0:64], tr2[:, :])
                else:
                    nc.vector.tensor_copy(A64[:, 64:128], tr2[:, :])

        # ---- W[q, 2s+comp] = psi_comp[128*idx_s + q]  (idx via sel input)
        psWa = ppool_st.tile([128, 4 * NBLK], F32, tag="stA")
        nc.tensor.matmul(psWa[:, :], A64a[:, :], sel_sb[:, :], start=True, stop=True)
        psWb = ppool_st.tile([128, 4 * NBLK], F32, tag="stB")
        nc.tensor.matmul(psWb[:, :], A64b[:, :], sel_sb[:, :], start=True, stop=True)
        W = cpool.tile([128, 8 * NBLK], BF16, tag="W")
        wv = W[:, :].rearrange("p (s two) -> two p s", two=2)
        nc.vector.tensor_copy(wv[0], psWa[:, :])
        nc.vector.tensor_copy(wv[1], psWb[:, :])

        # ---- matvec: 17 groups of 4 accumulating matmuls, drain to vout
        es_ps.close()   # release circuit PSUM banks
        ppool_mv = es.enter_context(tc.tile_pool(name="psmv", bufs=4,
                                                 space="PSUM"))
        vo = cpool.tile([2, NBLK * BLK], F32, tag="vo")
        for g in range(NBLK):
            PS = ppool_mv.tile([2, BLK], F32, tag="mv")
            for k in range(4):
                nc.tensor.matmul(PS[:, :], W[:, 8 * g + 2 * k:8 * g + 2 * k + 2],
                                 obs_t[g][:, BLK * k:BLK * (k + 1)],
                                 start=(k == 0), stop=(k == 3))
            if g % 2 == 0:
                nc.scalar.copy(vo[:, BLK * g:BLK * (g + 1)], PS[:, :])
            else:
                nc.vector.tensor_copy(vo[:, BLK * g:BLK * (g + 1)], PS[:, :])
        nc.sync.dma_start(vout.ap(), vo[:, :])

    nc.compile()
    return nc


def _get_program():
    if "nc" not in _CACHE:
        _CACHE["nc"] = _build_program()
        _CACHE["consts"] = _host_consts()
    return _CACHE["nc"], _CACHE["consts"]


def _make_in_maps(params, observable):
    nc, consts = _get_program()
    params = np.ascontiguousarray(params, dtype=np.float32)
    O = np.asarray(observable, dtype=np.float32)
    eye64 = np.eye(64, dtype=np.float32)
    in_maps = []
    for c in range(N_CORES):
        blocks = BLOCKS[NBLK * c:NBLK * (c + 1)]
        stream = np.empty((128, STREAM_COLS), dtype=ml_dtypes.bfloat16)
        idx = []
        for g, (R, C) in enumerate(blocks):
            Sb = O[BLK * R:BLK * (R + 1), BLK * C:BLK * (C + 1)]
            if R != C:
                Sb = Sb + O[BLK * C:BLK * (C + 1), BLK * R:BLK * (R + 1)].T
            Sb16 = Sb.astype(ml_dtypes.bfloat16)
            for k in range(4):
                stream[:, 2048 * g + BLK * k:2048 * g + BLK * (k + 1)] = \
                    Sb16[128 * k:128 * (k + 1), :]
                idx.append(4 * R + k)
        m = dict(consts)
        m["params"] = params
        m["obs"] = stream
        m["sel"] = np.ascontiguousarray(eye64[:, idx])
        in_maps.append(m)
    return nc, in_maps


def run(params, observable, trace=False):
    nc, in_maps = _make_in_maps(params, observable)
    res = run_bass_kernel_spmd(nc, in_maps, core_ids=list(range(N_CORES)),
                               trace=trace)
    a = np.asarray(res.results[0]["psi_re"], dtype=np.float64)
    b = np.asarray(res.results[0]["psi_im"], dtype=np.float64)
    loss = 0.0
    for c in range(N_CORES):
        v = np.asarray(res.results[c]["vout"], dtype=np.float64)
        for g, (R, C) in enumerate(BLOCKS[NBLK * c:NBLK * (c + 1)]):
            sl = slice(BLK * g, BLK * (g + 1))
            cl = slice(BLK * C, BLK * (C + 1))
            loss += v[0, sl] @ a[cl] + v[1, sl] @ b[cl]
    return np.float32(loss), res


def kernel(params, observable):
    loss, _ = run(params, observable, trace=False)
    return np.float32(loss)
